# revision 9
# baseline (speedup 1.0000x reference)
"""Trainium2 Bass kernel for nn_LossCR (segment-reduce + dual CE loss).

Strategy (data-parallel over N x H/2 -> 8 shards of 131072 pixels):
  Host packs, per core, a pixel-major fp8(e4m3) "combo" tensor
  (128 lanes, 1024 chunks, 150 cols) = [z(128) | preds(21) | 1]
  plus a small bf16 label tensor (128, 1024).
  Device, per 256-pixel double-chunk: one fp8 DoubleRow matmul
      S(21,150) += onehot(128,2,21)^T @ combo(128,2,150)
  PSUM-accumulated over all 512 double-chunks -> [sum_z | segsum_preds | counts].
  Onehot built on DVE (is_equal vs iota, fp8 out); CE pieces (exp from fp8,
  per-pixel sumexp, ln) on ACT/DVE with all 128 lanes busy.
  Host: sum 8 partial (22,152) outputs, tiny (21,) softmax math in f64.
  Note sum(preds) for the smoothing term = SP.sum() -- free from the matmul.
"""
import sys

sys.path.insert(0, "/opt/trn_rl_repo")
import numpy as np
import ml_dtypes
import concourse.bacc as bacc
import concourse.mybir as mybir
import concourse.tile as tile
from concourse import bass_utils
from concourse._compat import axon_active

f32 = mybir.dt.float32
bf16 = mybir.dt.bfloat16
f8 = mybir.dt.float8e4
AF = mybir.ActivationFunctionType
ALU = mybir.AluOpType
AX = mybir.AxisListType
PM = mybir.MatmulPerfMode

N, C, H, W, D = 4, 21, 512, 512, 128
NCORES = 8
PIX = N * H * W // NCORES      # 131072 pixels per core
CHUNKS = PIX // 128            # 1024
DCHUNKS = CHUNKS // 2          # 512 double-chunks (DoubleRow: 256 px each)
COLS = 150                     # z(128) | preds(21) | ones(1)
MC = 32                        # onehot cols per slab (DoubleRow needs 32-aligned)
G = 64                         # chunks per DMA group
NG = CHUNKS // G               # 16
LS = 0.1                       # label smoothing
LAMBDA_REG = 0.4

_nc_cache = None


def _build():
    global _nc_cache
    if _nc_cache is not None:
        return _nc_cache
    nc = bacc.Bacc("TRN2", target_bir_lowering=False, debug=not axon_active())
    cbd = nc.dram_tensor("combo", [128, CHUNKS * COLS], f8,
                         kind="ExternalInput").ap()
    labd = nc.dram_tensor("labels_pm", [128, CHUNKS], bf16,
                          kind="ExternalInput").ap()
    iotad = nc.dram_tensor("iota21", [128, MC], bf16, kind="ExternalInput").ap()
    outd = nc.dram_tensor("out", [22, COLS], f32, kind="ExternalOutput").ap()

    with tile.TileContext(nc) as tc:
        with tc.tile_pool(name="const", bufs=1) as cpool, \
             tc.tile_pool(name="work", bufs=4) as wpool, \
             tc.tile_pool(name="cb", bufs=4) as cbpool, \
             tc.tile_pool(name="ps", bufs=2, space="PSUM") as pspool, \
             tc.tile_pool(name="acc", bufs=1, space="PSUM") as apool:
            iota_sb = cpool.tile([128, MC], bf16, tag="iota_sb")
            nc.sync.dma_start(iota_sb[:], iotad)
            lab_sb = cpool.tile([128, CHUNKS], bf16, tag="lab_sb")
            nc.sync.dma_start(lab_sb[:], labd)
            ones_sb = cpool.tile([128, 1], f32, tag="ones_sb")
            nc.vector.memset(ones_sb[:], 1.0)
            red16 = cpool.tile([128, NG], f32, tag="red16")
            S_ps = apool.tile([MC, 150], f32, tag="S_ps")

            for g in range(NG):
                cb = cbpool.tile([128, G * COLS], f8, tag="cb")
                nc.sync.dma_start(cb[:], cbd[:, g * G * COLS:(g + 1) * G * COLS])
                cb_r = cb[:].rearrange("p (c m) -> p c m", m=COLS)
                # onehot(label) for the group's G*128 pixels, fp8 out
                oh = wpool.tile([128, G * MC], f8, tag="oh")
                oh_r = oh[:].rearrange("p (c k) -> p c k", k=MC)
                if g < 4:
                    # zero the 4 ring buffers once; pad cols 21:32 stay zero
                    nc.vector.memset(oh[:], 0.0)
                nc.vector.tensor_tensor(
                    oh_r[:, :, 0:C],
                    iota_sb[:, 0:C].unsqueeze(1).broadcast_to([128, G, C]),
                    lab_sb[:, g * G:(g + 1) * G].unsqueeze(2).broadcast_to(
                        [128, G, C]),
                    op=ALU.is_equal)
                # CE pieces: exp(preds) and per-pixel sumexp
                ex = wpool.tile([128, G * C], bf16, tag="ex")
                nc.scalar.activation(
                    ex[:].rearrange("p (c k) -> p c k", k=C),
                    cb_r[:, :, 128:149], AF.Exp)
                sxp = wpool.tile([128, G], f32, tag="sxp")
                nc.vector.tensor_reduce(
                    sxp[:],
                    ex[:].rearrange("p (c k) -> p c k", k=C),
                    axis=AX.X, op=ALU.add)
                lse_g = wpool.tile([128, G], f32, tag="lse_g")
                nc.scalar.activation(lse_g[:], sxp[:], AF.Ln)
                nc.vector.tensor_reduce(red16[:, g:g + 1], lse_g[:],
                                        axis=AX.X, op=ALU.add)
                # segment sums: S += oh^T @ [z | preds | 1], 2 chunks per matmul
                for i in range(G // 2):
                    dc = g * (G // 2) + i
                    nc.tensor.matmul(
                        S_ps[:],
                        oh_r[:, 2 * i:2 * i + 2, :],
                        cb_r[:, 2 * i:2 * i + 2, 0:150],
                        start=(dc == 0), stop=(dc == DCHUNKS - 1),
                        perf_mode=PM.DoubleRow)

            # --- epilogue: slse = sum over the per-group partials
            red = cpool.tile([128, 1], f32, tag="red")
            nc.vector.tensor_reduce(red[:], red16[:], axis=AX.X, op=ALU.add)
            fin_ps = pspool.tile([1, 1], f32, tag="fin_ps", bufs=1)
            nc.tensor.matmul(fin_ps[:], ones_sb[:], red[:], start=True, stop=True)
            row = cpool.tile([1, 2], f32, tag="row")
            nc.vector.memset(row[:], 0.0)
            nc.scalar.copy(row[:, 0:1], fin_ps[:])
            S_sb = cpool.tile([C, 150], f32, tag="S_sb")
            nc.scalar.copy(S_sb[:], S_ps[0:C, :])
            nc.sync.dma_start(outd[0:C, 0:150], S_sb[:])
            nc.sync.dma_start(outd[C:C + 1, 0:2], row[:])

    nc.compile()
    _nc_cache = nc
    return nc


_F8 = ml_dtypes.float8_e4m3
_BF16 = ml_dtypes.bfloat16
_IOTA = np.tile(np.arange(32, dtype=np.float32), (128, 1)).astype(_BF16)


def _make_in_maps(preds, labels, z, W_star):
    in_maps = []
    for i in range(NCORES):
        n, h0 = i // 2, (i % 2) * (H // 2)
        # pixel p = h*512 + w -> chunk = h*4 + w//128, lane = w%128
        zc = z[n, :, h0:h0 + H // 2, :].reshape(D, 256, 4, 128)
        zc = zc.transpose(3, 1, 2, 0).reshape(128, CHUNKS, D)
        pc = preds[n, :, h0:h0 + H // 2, :].reshape(C, 256, 4, 128)
        pc = pc.transpose(3, 1, 2, 0).reshape(128, CHUNKS, C)
        lc = labels[n, h0:h0 + H // 2, :].reshape(256, 4, 128)
        lc = lc.transpose(2, 0, 1).reshape(128, CHUNKS)
        combo = np.zeros((128, CHUNKS, COLS), dtype=_F8)
        combo[:, :, 0:D] = zc.astype(_F8)
        combo[:, :, D:D + C] = pc.astype(_F8)
        combo[:, :, 149] = np.float32(1.0)
        in_maps.append(dict(combo=combo.reshape(128, CHUNKS * COLS),
                            labels_pm=lc.astype(_BF16),
                            iota21=_IOTA))
    return in_maps


def _combine(outs, W_star):
    """outs: list of 8 arrays (22,152) -> final scalar loss (float32 0-d)."""
    tot = np.sum([o.astype(np.float64) for o in outs], axis=0)
    S_z = tot[0:C, 0:D]
    SP = tot[0:C, D:D + C]
    cnt = tot[0:C, 149]
    slse = tot[C, 0]
    ssx = SP.sum()
    npix = max(cnt.sum(), 1.0)
    sem = (slse - (1.0 - LS) * np.trace(SP) - (LS / C) * ssx) / npix
    Zbar = np.where(cnt[:, None] > 0, S_z / np.maximum(cnt, 1.0)[:, None], 0.0)
    logits = Zbar @ W_star.astype(np.float64)
    m = logits.max(axis=1, keepdims=True)
    lse_r = m[:, 0] + np.log(np.exp(logits - m).sum(axis=1))
    lcr = np.mean(lse_r - (1.0 - LS) * np.diag(logits)
                  - (LS / C) * logits.sum(axis=1))
    return np.float32(LAMBDA_REG * lcr + sem)


def kernel(preds, labels, labels_depth, z, W_star):
    nc = _build()
    in_maps = _make_in_maps(preds, labels, z, W_star)
    res = bass_utils.run_bass_kernel_spmd(nc, in_maps,
                                          core_ids=list(range(NCORES)))
    return _combine([r["out"] for r in res.results], W_star)


if __name__ == "__main__":
    rng = np.random.default_rng(0)
    preds = rng.standard_normal((N, C, H, W), dtype=np.float32)
    labels = rng.integers(0, C, size=(N, H, W)).astype(np.int32)
    ld = rng.standard_normal((N, H, W), dtype=np.float32)
    z = rng.standard_normal((N, D, H, W), dtype=np.float32)
    Wst = rng.standard_normal((D, C), dtype=np.float32) * 0.3
    print("loss:", kernel(preds, labels, ld, z, Wst))


# revision 12
# speedup vs baseline: 1.0909x; 1.0909x over previous
"""Trainium2 Bass kernel for nn_LossCR (segment-reduce + dual CE loss).

Strategy (data-parallel over N x H/2 -> 8 shards of 131072 pixels):
  Host packs, per core, a pixel-major fp8(e4m3) "combo" tensor
  (128 lanes, 1024 chunks, 150 cols) = [z(128) | preds(21) | 1]
  plus a small bf16 label tensor (128, 1024).
  Device, per 256-pixel double-chunk: one fp8 DoubleRow matmul
      S(21,150) += onehot(128,2,21)^T @ combo(128,2,150)
  PSUM-accumulated over all 512 double-chunks -> [sum_z | segsum_preds | counts].
  Onehot built on DVE (is_equal vs iota, fp8 out); CE pieces (exp from fp8,
  per-pixel sumexp, ln) on ACT/DVE with all 128 lanes busy.
  Host: sum 8 partial (22,152) outputs, tiny (21,) softmax math in f64.
  Note sum(preds) for the smoothing term = SP.sum() -- free from the matmul.
"""
import sys

sys.path.insert(0, "/opt/trn_rl_repo")
import numpy as np
import ml_dtypes
import concourse.bacc as bacc
import concourse.mybir as mybir
import concourse.tile as tile
from concourse import bass_utils
from concourse._compat import axon_active

f32 = mybir.dt.float32
bf16 = mybir.dt.bfloat16
f8 = mybir.dt.float8e4
AF = mybir.ActivationFunctionType
ALU = mybir.AluOpType
AX = mybir.AxisListType
PM = mybir.MatmulPerfMode

N, C, H, W, D = 4, 21, 512, 512, 128
NCORES = 8
PIX = N * H * W // NCORES      # 131072 pixels per core
CHUNKS = PIX // 128            # 1024
DCHUNKS = CHUNKS // 2          # 512 double-chunks (DoubleRow: 256 px each)
COLS = 150                     # z(128) | preds(21) | ones(1)
MC = 32                        # onehot cols per slab (DoubleRow needs 32-aligned)
G = 64                         # chunks per DMA group
NG = CHUNKS // G               # 16
LS = 0.1                       # label smoothing
LAMBDA_REG = 0.4

_nc_cache = None


def _build():
    global _nc_cache
    if _nc_cache is not None:
        return _nc_cache
    nc = bacc.Bacc("TRN2", target_bir_lowering=False, debug=not axon_active())
    cbd = nc.dram_tensor("combo", [128, CHUNKS * COLS], f8,
                         kind="ExternalInput").ap()
    labd = nc.dram_tensor("labels_pm", [128, CHUNKS], bf16,
                          kind="ExternalInput").ap()
    iotad = nc.dram_tensor("iota21", [128, MC], bf16, kind="ExternalInput").ap()
    outd = nc.dram_tensor("out", [22, COLS], f32, kind="ExternalOutput").ap()
    out2d = nc.dram_tensor("slse_row", [1, 1], f32,
                           kind="ExternalOutput").ap()

    with tile.TileContext(nc) as tc:
        with tc.tile_pool(name="const", bufs=1) as cpool, \
             tc.tile_pool(name="work", bufs=4) as wpool, \
             tc.tile_pool(name="cb", bufs=4) as cbpool, \
             tc.tile_pool(name="ps", bufs=2, space="PSUM") as pspool, \
             tc.tile_pool(name="acc", bufs=1, space="PSUM") as apool:
            iota_sb = cpool.tile([128, MC], bf16, tag="iota_sb")
            nc.sync.dma_start(iota_sb[:], iotad)
            lab_sb = cpool.tile([128, CHUNKS], bf16, tag="lab_sb")
            nc.sync.dma_start(lab_sb[:], labd)
            ones_sb = cpool.tile([128, 1], f32, tag="ones_sb")
            nc.vector.memset(ones_sb[:], 1.0)
            sumexp_buf = cpool.tile([128, CHUNKS], f32, tag="sumexp_buf")
            S_ps = apool.tile([MC, 150], f32, tag="S_ps")

            for g in range(NG):
                cb = cbpool.tile([128, G * COLS], f8, tag="cb")
                nc.sync.dma_start(cb[:], cbd[:, g * G * COLS:(g + 1) * G * COLS])
                cb_r = cb[:].rearrange("p (c m) -> p c m", m=COLS)
                # onehot(label) for the group's G*128 pixels, fp8 out
                oh = wpool.tile([128, G * MC], f8, tag="oh")
                oh_r = oh[:].rearrange("p (c k) -> p c k", k=MC)
                if g < 4:
                    # zero the 4 ring buffers once; pad cols 21:32 stay zero
                    nc.vector.memset(oh[:], 0.0)
                nc.vector.tensor_tensor(
                    oh_r[:, :, 0:C],
                    iota_sb[:, 0:C].unsqueeze(1).broadcast_to([128, G, C]),
                    lab_sb[:, g * G:(g + 1) * G].unsqueeze(2).broadcast_to(
                        [128, G, C]),
                    op=ALU.is_equal)
                # CE pieces: exp(preds) and per-pixel sumexp
                ex = wpool.tile([128, G * C], bf16, tag="ex")
                nc.scalar.activation(
                    ex[:].rearrange("p (c k) -> p c k", k=C),
                    cb_r[:, :, 128:149], AF.Exp)
                nc.vector.tensor_reduce(
                    sumexp_buf[:, g * G:(g + 1) * G],
                    ex[:].rearrange("p (c k) -> p c k", k=C),
                    axis=AX.X, op=ALU.add)
                # segment sums: S += oh^T @ [z | preds | 1], 2 chunks per matmul
                for i in range(G // 2):
                    dc = g * (G // 2) + i
                    nc.tensor.matmul(
                        S_ps[:],
                        oh_r[:, 2 * i:2 * i + 2, :],
                        cb_r[:, 2 * i:2 * i + 2, 0:150],
                        start=(dc == 0), stop=(dc == DCHUNKS - 1),
                        perf_mode=PM.DoubleRow)

            # --- epilogue: slse = sum(ln(sumexp)) over all pixels
            lse = cpool.tile([128, CHUNKS], f32, tag="lse")
            nc.scalar.activation(lse[:], sumexp_buf[:], AF.Ln)
            red = cpool.tile([128, 1], f32, tag="red")
            nc.vector.tensor_reduce(red[:], lse[:], axis=AX.X, op=ALU.add)
            fin_ps = pspool.tile([1, 1], f32, tag="fin_ps", bufs=1)
            nc.tensor.matmul(fin_ps[:], ones_sb[:], red[:], start=True, stop=True)
            fin_sb = cpool.tile([1, 1], f32, tag="fin_sb")
            nc.scalar.copy(fin_sb[:], fin_ps[:])
            nc.sync.dma_start(out2d, fin_sb[:])
            S_sb = cpool.tile([C, 150], f32, tag="S_sb")
            nc.scalar.copy(S_sb[:], S_ps[0:C, :])
            nc.sync.dma_start(outd[0:C, 0:150], S_sb[:])

    nc.compile()
    _nc_cache = nc
    return nc


_F8 = ml_dtypes.float8_e4m3
_BF16 = ml_dtypes.bfloat16
_IOTA = np.tile(np.arange(32, dtype=np.float32), (128, 1)).astype(_BF16)


def _make_in_maps(preds, labels, z, W_star):
    in_maps = []
    for i in range(NCORES):
        n, h0 = i // 2, (i % 2) * (H // 2)
        # pixel p = h*512 + w -> chunk = h*4 + w//128, lane = w%128
        zc = z[n, :, h0:h0 + H // 2, :].reshape(D, 256, 4, 128)
        zc = zc.transpose(3, 1, 2, 0).reshape(128, CHUNKS, D)
        pc = preds[n, :, h0:h0 + H // 2, :].reshape(C, 256, 4, 128)
        pc = pc.transpose(3, 1, 2, 0).reshape(128, CHUNKS, C)
        lc = labels[n, h0:h0 + H // 2, :].reshape(256, 4, 128)
        lc = lc.transpose(2, 0, 1).reshape(128, CHUNKS)
        combo = np.zeros((128, CHUNKS, COLS), dtype=_F8)
        combo[:, :, 0:D] = zc.astype(_F8)
        combo[:, :, D:D + C] = pc.astype(_F8)
        combo[:, :, 149] = np.float32(1.0)
        in_maps.append(dict(combo=combo.reshape(128, CHUNKS * COLS),
                            labels_pm=lc.astype(_BF16),
                            iota21=_IOTA))
    return in_maps


def _combine(outs, slse_rows, W_star):
    """outs: 8x(22,150) S blocks + 8x(1,1024) lse rows -> scalar loss."""
    tot = np.sum([o.astype(np.float64) for o in outs], axis=0)
    S_z = tot[0:C, 0:D]
    SP = tot[0:C, D:D + C]
    cnt = tot[0:C, 149]
    slse = np.sum([r.astype(np.float64).sum() for r in slse_rows])
    ssx = SP.sum()
    npix = max(cnt.sum(), 1.0)
    sem = (slse - (1.0 - LS) * np.trace(SP) - (LS / C) * ssx) / npix
    Zbar = np.where(cnt[:, None] > 0, S_z / np.maximum(cnt, 1.0)[:, None], 0.0)
    logits = Zbar @ W_star.astype(np.float64)
    m = logits.max(axis=1, keepdims=True)
    lse_r = m[:, 0] + np.log(np.exp(logits - m).sum(axis=1))
    lcr = np.mean(lse_r - (1.0 - LS) * np.diag(logits)
                  - (LS / C) * logits.sum(axis=1))
    return np.float32(LAMBDA_REG * lcr + sem)


def kernel(preds, labels, labels_depth, z, W_star):
    nc = _build()
    in_maps = _make_in_maps(preds, labels, z, W_star)
    res = bass_utils.run_bass_kernel_spmd(nc, in_maps,
                                          core_ids=list(range(NCORES)))
    return _combine([r["out"] for r in res.results],
                    [r["slse_row"] for r in res.results], W_star)


if __name__ == "__main__":
    rng = np.random.default_rng(0)
    preds = rng.standard_normal((N, C, H, W), dtype=np.float32)
    labels = rng.integers(0, C, size=(N, H, W)).astype(np.int32)
    ld = rng.standard_normal((N, H, W), dtype=np.float32)
    z = rng.standard_normal((N, D, H, W), dtype=np.float32)
    Wst = rng.standard_normal((D, C), dtype=np.float32) * 0.3
    print("loss:", kernel(preds, labels, ld, z, Wst))


# revision 14
# speedup vs baseline: 1.1171x; 1.0241x over previous
"""Trainium2 Bass kernel for nn_LossCR (segment-reduce + dual CE loss).

Strategy (data-parallel over N x H/2 -> 8 shards of 131072 pixels):
  Host packs, per core, a pixel-major fp8(e4m3) "combo" tensor
  (128 lanes, 1024 chunks, 150 cols) = [z(128) | preds(21) | 1]
  plus a small bf16 label tensor (128, 1024).
  Device, per 256-pixel double-chunk: one fp8 DoubleRow matmul
      S(21,150) += onehot(128,2,21)^T @ combo(128,2,150)
  PSUM-accumulated over all 512 double-chunks -> [sum_z | segsum_preds | counts].
  Onehot built on DVE (is_equal vs iota, fp8 out); CE pieces (exp from fp8,
  per-pixel sumexp, ln) on ACT/DVE with all 128 lanes busy.
  Host: sum 8 partial (22,152) outputs, tiny (21,) softmax math in f64.
  Note sum(preds) for the smoothing term = SP.sum() -- free from the matmul.
"""
import sys

sys.path.insert(0, "/opt/trn_rl_repo")
import numpy as np
import ml_dtypes
import concourse.bacc as bacc
import concourse.mybir as mybir
import concourse.tile as tile
from concourse import bass_utils
from concourse._compat import axon_active

f32 = mybir.dt.float32
bf16 = mybir.dt.bfloat16
f8 = mybir.dt.float8e4
AF = mybir.ActivationFunctionType
ALU = mybir.AluOpType
AX = mybir.AxisListType
PM = mybir.MatmulPerfMode

N, C, H, W, D = 4, 21, 512, 512, 128
NCORES = 8
PIX = N * H * W // NCORES      # 131072 pixels per core
CHUNKS = PIX // 128            # 1024
DCHUNKS = CHUNKS // 2          # 512 double-chunks (DoubleRow: 256 px each)
COLS = 150                     # z(128) | preds(21) | ones(1)
MC = 32                        # onehot cols per slab (DoubleRow needs 32-aligned)
G = 64                         # chunks per DMA group
NG = CHUNKS // G               # 16
LS = 0.1                       # label smoothing
LAMBDA_REG = 0.4

_nc_cache = None


def _build():
    global _nc_cache
    if _nc_cache is not None:
        return _nc_cache
    nc = bacc.Bacc("TRN2", target_bir_lowering=False, debug=not axon_active())
    cbd = nc.dram_tensor("combo", [128, CHUNKS * COLS], f8,
                         kind="ExternalInput").ap()
    labd = nc.dram_tensor("labels_pm", [128, CHUNKS], bf16,
                          kind="ExternalInput").ap()
    iotad = nc.dram_tensor("iota21", [128, MC], bf16, kind="ExternalInput").ap()
    outd = nc.dram_tensor("out", [22, COLS], f32, kind="ExternalOutput").ap()
    out2d = nc.dram_tensor("slse_row", [1, 1], f32,
                           kind="ExternalOutput").ap()

    with tile.TileContext(nc) as tc:
        with tc.tile_pool(name="const", bufs=1) as cpool, \
             tc.tile_pool(name="work", bufs=4) as wpool, \
             tc.tile_pool(name="cb", bufs=6) as cbpool, \
             tc.tile_pool(name="ps", bufs=2, space="PSUM") as pspool, \
             tc.tile_pool(name="acc", bufs=1, space="PSUM") as apool:
            iota_sb = cpool.tile([128, MC], bf16, tag="iota_sb")
            nc.sync.dma_start(iota_sb[:], iotad)
            lab_sb = cpool.tile([128, CHUNKS], bf16, tag="lab_sb")
            nc.sync.dma_start(lab_sb[:], labd)
            ones_sb = cpool.tile([128, 1], f32, tag="ones_sb")
            nc.vector.memset(ones_sb[:], 1.0)
            sumexp_buf = cpool.tile([128, CHUNKS], f32, tag="sumexp_buf")
            S_ps = apool.tile([MC, 150], f32, tag="S_ps")

            for g in range(NG):
                cb = cbpool.tile([128, G * COLS], f8, tag="cb")
                nc.sync.dma_start(cb[:], cbd[:, g * G * COLS:(g + 1) * G * COLS])
                cb_r = cb[:].rearrange("p (c m) -> p c m", m=COLS)
                # onehot(label) for the group's G*128 pixels, fp8 out
                # pad cols 21:32 are left as garbage: each stationary column
                # only feeds its own S_ps row, and rows 21:31 are discarded
                oh = wpool.tile([128, G * MC], f8, tag="oh")
                oh_r = oh[:].rearrange("p (c k) -> p c k", k=MC)
                nc.vector.tensor_tensor(
                    oh_r[:, :, 0:C],
                    iota_sb[:, 0:C].unsqueeze(1).broadcast_to([128, G, C]),
                    lab_sb[:, g * G:(g + 1) * G].unsqueeze(2).broadcast_to(
                        [128, G, C]),
                    op=ALU.is_equal)
                # CE pieces: exp(preds) and per-pixel sumexp
                ex = wpool.tile([128, G * C], bf16, tag="ex")
                nc.scalar.activation(
                    ex[:].rearrange("p (c k) -> p c k", k=C),
                    cb_r[:, :, 128:149], AF.Exp)
                nc.vector.tensor_reduce(
                    sumexp_buf[:, g * G:(g + 1) * G],
                    ex[:].rearrange("p (c k) -> p c k", k=C),
                    axis=AX.X, op=ALU.add)
                # segment sums: S += oh^T @ [z | preds | 1], 2 chunks per matmul
                for i in range(G // 2):
                    dc = g * (G // 2) + i
                    nc.tensor.matmul(
                        S_ps[:],
                        oh_r[:, 2 * i:2 * i + 2, :],
                        cb_r[:, 2 * i:2 * i + 2, 0:150],
                        start=(dc == 0), stop=(dc == DCHUNKS - 1),
                        perf_mode=PM.DoubleRow)

            # --- epilogue: slse = sum(ln(sumexp)) over all pixels
            lse = cpool.tile([128, CHUNKS], f32, tag="lse")
            nc.scalar.activation(lse[:], sumexp_buf[:], AF.Ln)
            red = cpool.tile([128, 1], f32, tag="red")
            nc.vector.tensor_reduce(red[:], lse[:], axis=AX.X, op=ALU.add)
            fin_ps = pspool.tile([1, 1], f32, tag="fin_ps", bufs=1)
            nc.tensor.matmul(fin_ps[:], ones_sb[:], red[:], start=True, stop=True)
            fin_sb = cpool.tile([1, 1], f32, tag="fin_sb")
            nc.scalar.copy(fin_sb[:], fin_ps[:])
            nc.sync.dma_start(out2d, fin_sb[:])
            S_sb = cpool.tile([C, 150], f32, tag="S_sb")
            nc.scalar.copy(S_sb[:], S_ps[0:C, :])
            nc.sync.dma_start(outd[0:C, 0:150], S_sb[:])

    nc.compile()
    _nc_cache = nc
    return nc


_F8 = ml_dtypes.float8_e4m3
_BF16 = ml_dtypes.bfloat16
_IOTA = np.tile(np.arange(32, dtype=np.float32), (128, 1)).astype(_BF16)


def _make_in_maps(preds, labels, z, W_star):
    in_maps = []
    for i in range(NCORES):
        n, h0 = i // 2, (i % 2) * (H // 2)
        # pixel p = h*512 + w -> chunk = h*4 + w//128, lane = w%128
        zc = z[n, :, h0:h0 + H // 2, :].reshape(D, 256, 4, 128)
        zc = zc.transpose(3, 1, 2, 0).reshape(128, CHUNKS, D)
        pc = preds[n, :, h0:h0 + H // 2, :].reshape(C, 256, 4, 128)
        pc = pc.transpose(3, 1, 2, 0).reshape(128, CHUNKS, C)
        lc = labels[n, h0:h0 + H // 2, :].reshape(256, 4, 128)
        lc = lc.transpose(2, 0, 1).reshape(128, CHUNKS)
        combo = np.zeros((128, CHUNKS, COLS), dtype=_F8)
        combo[:, :, 0:D] = zc.astype(_F8)
        combo[:, :, D:D + C] = pc.astype(_F8)
        combo[:, :, 149] = np.float32(1.0)
        in_maps.append(dict(combo=combo.reshape(128, CHUNKS * COLS),
                            labels_pm=lc.astype(_BF16),
                            iota21=_IOTA))
    return in_maps


def _combine(outs, slse_rows, W_star):
    """outs: 8x(22,150) S blocks + 8x(1,1024) lse rows -> scalar loss."""
    tot = np.sum([o.astype(np.float64) for o in outs], axis=0)
    S_z = tot[0:C, 0:D]
    SP = tot[0:C, D:D + C]
    cnt = tot[0:C, 149]
    slse = np.sum([r.astype(np.float64).sum() for r in slse_rows])
    ssx = SP.sum()
    npix = max(cnt.sum(), 1.0)
    sem = (slse - (1.0 - LS) * np.trace(SP) - (LS / C) * ssx) / npix
    Zbar = np.where(cnt[:, None] > 0, S_z / np.maximum(cnt, 1.0)[:, None], 0.0)
    logits = Zbar @ W_star.astype(np.float64)
    m = logits.max(axis=1, keepdims=True)
    lse_r = m[:, 0] + np.log(np.exp(logits - m).sum(axis=1))
    lcr = np.mean(lse_r - (1.0 - LS) * np.diag(logits)
                  - (LS / C) * logits.sum(axis=1))
    return np.float32(LAMBDA_REG * lcr + sem)


def kernel(preds, labels, labels_depth, z, W_star):
    nc = _build()
    in_maps = _make_in_maps(preds, labels, z, W_star)
    res = bass_utils.run_bass_kernel_spmd(nc, in_maps,
                                          core_ids=list(range(NCORES)))
    return _combine([r["out"] for r in res.results],
                    [r["slse_row"] for r in res.results], W_star)


if __name__ == "__main__":
    rng = np.random.default_rng(0)
    preds = rng.standard_normal((N, C, H, W), dtype=np.float32)
    labels = rng.integers(0, C, size=(N, H, W)).astype(np.int32)
    ld = rng.standard_normal((N, H, W), dtype=np.float32)
    z = rng.standard_normal((N, D, H, W), dtype=np.float32)
    Wst = rng.standard_normal((D, C), dtype=np.float32) * 0.3
    print("loss:", kernel(preds, labels, ld, z, Wst))


# revision 15
# speedup vs baseline: 1.2115x; 1.0845x over previous
"""Trainium2 Bass kernel for nn_LossCR (segment-reduce + dual CE loss).

Strategy (data-parallel over N x H/2 -> 8 shards of 131072 pixels):
  Host packs, per core, a pixel-major fp8(e4m3) "combo" tensor
  (128 lanes, 1024 chunks, 150 cols) = [z(128) | preds(21) | 1]
  plus a small bf16 label tensor (128, 1024).
  Device, per 256-pixel double-chunk: one fp8 DoubleRow matmul
      S(21,150) += onehot(128,2,21)^T @ combo(128,2,150)
  PSUM-accumulated over all 512 double-chunks -> [sum_z | segsum_preds | counts].
  Onehot built on DVE (is_equal vs iota, fp8 out, 32-col slabs with junk
  pads feeding only discarded S rows); CE pieces (exp from fp8, per-pixel
  sumexp, ln) on ACT/DVE with all 128 lanes busy.
  Host: sum 8 partial (22,150) S blocks + 8 slse scalars, tiny (21,)
  softmax math in f64.
  Note sum(preds) for the smoothing term = SP.sum() -- free from the matmul.
"""
import sys

sys.path.insert(0, "/opt/trn_rl_repo")
import numpy as np
import ml_dtypes
import concourse.bacc as bacc
import concourse.mybir as mybir
import concourse.tile as tile
from concourse import bass_utils
from concourse._compat import axon_active

f32 = mybir.dt.float32
bf16 = mybir.dt.bfloat16
f8 = mybir.dt.float8e4
AF = mybir.ActivationFunctionType
ALU = mybir.AluOpType
AX = mybir.AxisListType
PM = mybir.MatmulPerfMode

N, C, H, W, D = 4, 21, 512, 512, 128
NCORES = 8
PIX = N * H * W // NCORES      # 131072 pixels per core
CHUNKS = PIX // 128            # 1024
DCHUNKS = CHUNKS // 2          # 512 double-chunks (DoubleRow: 256 px each)
COLS = 150                     # z(128) | preds(21) | ones(1)
MC = 32                        # onehot cols per slab (DoubleRow needs 32-aligned)
G = 64                         # chunks per DMA group
NG = CHUNKS // G               # 16
LS = 0.1                       # label smoothing
LAMBDA_REG = 0.4

_nc_cache = None


def _build():
    global _nc_cache
    if _nc_cache is not None:
        return _nc_cache
    nc = bacc.Bacc("TRN2", target_bir_lowering=False, debug=not axon_active())
    cbd = nc.dram_tensor("combo", [128, CHUNKS * COLS], f8,
                         kind="ExternalInput").ap()
    labd = nc.dram_tensor("labels_pm", [128, CHUNKS], bf16,
                          kind="ExternalInput").ap()
    iotad = nc.dram_tensor("iota21", [128, MC], bf16, kind="ExternalInput").ap()
    outd = nc.dram_tensor("out", [22, COLS], f32, kind="ExternalOutput").ap()
    out2d = nc.dram_tensor("slse_row", [1, 1], f32,
                           kind="ExternalOutput").ap()

    with tile.TileContext(nc) as tc:
        with tc.tile_pool(name="const", bufs=1) as cpool, \
             tc.tile_pool(name="work", bufs=4) as wpool, \
             tc.tile_pool(name="cb", bufs=6) as cbpool, \
             tc.tile_pool(name="ps", bufs=2, space="PSUM") as pspool, \
             tc.tile_pool(name="acc", bufs=1, space="PSUM") as apool:
            iota_sb = cpool.tile([128, MC], bf16, tag="iota_sb")
            nc.sync.dma_start(iota_sb[:], iotad)
            lab_sb = cpool.tile([128, CHUNKS], bf16, tag="lab_sb")
            nc.sync.dma_start(lab_sb[:], labd)
            ones_sb = cpool.tile([128, 1], f32, tag="ones_sb")
            nc.vector.memset(ones_sb[:], 1.0)
            sumexp_buf = cpool.tile([128, CHUNKS], f32, tag="sumexp_buf")
            S_ps = apool.tile([MC, 150], f32, tag="S_ps")

            for g in range(NG):
                cb = cbpool.tile([128, G * COLS], f8, tag="cb")
                nc.sync.dma_start(cb[:], cbd[:, g * G * COLS:(g + 1) * G * COLS])
                cb_r = cb[:].rearrange("p (c m) -> p c m", m=COLS)
                # onehot(label) for the group's G*128 pixels, fp8 out
                # pad cols 21:32 are left as garbage: each stationary column
                # only feeds its own S_ps row, and rows 21:31 are discarded
                oh = wpool.tile([128, G * MC], f8, tag="oh")
                oh_r = oh[:].rearrange("p (c k) -> p c k", k=MC)
                nc.vector.tensor_tensor(
                    oh_r[:, :, 0:C],
                    iota_sb[:, 0:C].unsqueeze(1).broadcast_to([128, G, C]),
                    lab_sb[:, g * G:(g + 1) * G].unsqueeze(2).broadcast_to(
                        [128, G, C]),
                    op=ALU.is_equal)
                # CE pieces: exp(preds) and per-pixel sumexp
                ex = wpool.tile([128, G * C], bf16, tag="ex")
                nc.scalar.activation(
                    ex[:].rearrange("p (c k) -> p c k", k=C),
                    cb_r[:, :, 128:149], AF.Exp)
                nc.vector.tensor_reduce(
                    sumexp_buf[:, g * G:(g + 1) * G],
                    ex[:].rearrange("p (c k) -> p c k", k=C),
                    axis=AX.X, op=ALU.add)
                # segment sums: S += oh^T @ [z | preds | 1], 2 chunks per matmul
                for i in range(G // 2):
                    dc = g * (G // 2) + i
                    nc.tensor.matmul(
                        S_ps[:],
                        oh_r[:, 2 * i:2 * i + 2, :],
                        cb_r[:, 2 * i:2 * i + 2, 0:150],
                        start=(dc == 0), stop=(dc == DCHUNKS - 1),
                        perf_mode=PM.DoubleRow)

            # --- epilogue: slse = sum(ln(sumexp)) over all pixels
            lse = cpool.tile([128, CHUNKS], f32, tag="lse")
            nc.scalar.activation(lse[:], sumexp_buf[:], AF.Ln)
            red = cpool.tile([128, 1], f32, tag="red")
            nc.vector.tensor_reduce(red[:], lse[:], axis=AX.X, op=ALU.add)
            fin_ps = pspool.tile([1, 1], f32, tag="fin_ps", bufs=1)
            nc.tensor.matmul(fin_ps[:], ones_sb[:], red[:], start=True, stop=True)
            fin_sb = cpool.tile([1, 1], f32, tag="fin_sb")
            nc.scalar.copy(fin_sb[:], fin_ps[:])
            nc.sync.dma_start(out2d, fin_sb[:])
            S_sb = cpool.tile([C, 150], f32, tag="S_sb")
            nc.scalar.copy(S_sb[:], S_ps[0:C, :])
            nc.sync.dma_start(outd[0:C, 0:150], S_sb[:])

    nc.compile()
    _nc_cache = nc
    return nc


_F8 = ml_dtypes.float8_e4m3
_BF16 = ml_dtypes.bfloat16
_IOTA = np.tile(np.arange(32, dtype=np.float32), (128, 1)).astype(_BF16)


def _make_in_maps(preds, labels, z, W_star):
    in_maps = []
    for i in range(NCORES):
        n, h0 = i // 2, (i % 2) * (H // 2)
        # pixel p = h*512 + w -> chunk = h*4 + w//128, lane = w%128
        zc = z[n, :, h0:h0 + H // 2, :].reshape(D, 256, 4, 128)
        zc = zc.transpose(3, 1, 2, 0).reshape(128, CHUNKS, D)
        pc = preds[n, :, h0:h0 + H // 2, :].reshape(C, 256, 4, 128)
        pc = pc.transpose(3, 1, 2, 0).reshape(128, CHUNKS, C)
        lc = labels[n, h0:h0 + H // 2, :].reshape(256, 4, 128)
        lc = lc.transpose(2, 0, 1).reshape(128, CHUNKS)
        combo = np.zeros((128, CHUNKS, COLS), dtype=_F8)
        combo[:, :, 0:D] = zc.astype(_F8)
        combo[:, :, D:D + C] = pc.astype(_F8)
        combo[:, :, 149] = np.float32(1.0)
        in_maps.append(dict(combo=combo.reshape(128, CHUNKS * COLS),
                            labels_pm=lc.astype(_BF16),
                            iota21=_IOTA))
    return in_maps


def _combine(outs, slse_rows, W_star):
    """outs: 8x(22,150) S blocks + 8x(1,1024) lse rows -> scalar loss."""
    tot = np.sum([o.astype(np.float64) for o in outs], axis=0)
    S_z = tot[0:C, 0:D]
    SP = tot[0:C, D:D + C]
    cnt = tot[0:C, 149]
    slse = np.sum([r.astype(np.float64).sum() for r in slse_rows])
    ssx = SP.sum()
    npix = max(cnt.sum(), 1.0)
    sem = (slse - (1.0 - LS) * np.trace(SP) - (LS / C) * ssx) / npix
    Zbar = np.where(cnt[:, None] > 0, S_z / np.maximum(cnt, 1.0)[:, None], 0.0)
    logits = Zbar @ W_star.astype(np.float64)
    m = logits.max(axis=1, keepdims=True)
    lse_r = m[:, 0] + np.log(np.exp(logits - m).sum(axis=1))
    lcr = np.mean(lse_r - (1.0 - LS) * np.diag(logits)
                  - (LS / C) * logits.sum(axis=1))
    return np.float32(LAMBDA_REG * lcr + sem)


def kernel(preds, labels, labels_depth, z, W_star):
    nc = _build()
    in_maps = _make_in_maps(preds, labels, z, W_star)
    res = bass_utils.run_bass_kernel_spmd(nc, in_maps,
                                          core_ids=list(range(NCORES)))
    return _combine([r["out"] for r in res.results],
                    [r["slse_row"] for r in res.results], W_star)


if __name__ == "__main__":
    rng = np.random.default_rng(0)
    preds = rng.standard_normal((N, C, H, W), dtype=np.float32)
    labels = rng.integers(0, C, size=(N, H, W)).astype(np.int32)
    ld = rng.standard_normal((N, H, W), dtype=np.float32)
    z = rng.standard_normal((N, D, H, W), dtype=np.float32)
    Wst = rng.standard_normal((D, C), dtype=np.float32) * 0.3
    print("loss:", kernel(preds, labels, ld, z, Wst))
